# revision 1
# baseline (speedup 1.0000x reference)
"""Multi-head attention block (nn_Attention) on 8 Trainium2 NeuronCores.

Reference computation (fp32):
    qkv = x @ w_qkv;  q,k,v = split(qkv);  per-head softmax(q k^T / sqrt(d)) v
    out = concat_heads @ w_out + b_out
Shapes: x [4, 2048, 1024], w_qkv [1024, 3072], w_out [1024, 1024], b_out [1024].

Sharding: DP over batch (4) x TP over head-groups (2) = 8 cores.
Core c handles batch c//2 and heads [8*(c%2), 8*(c%2)+8). Each core computes a
partial output projection over its 8 heads; the host sums the two partials per
batch and adds b_out (the unshard/gather step). No on-device collectives.

Per-core kernel:
  head:  x -> x^T in SBUF (PE transpose); V = x w_v (+ones col -> V_aug, fp16);
         Q^T/K^T chunks for head-pair 0 (fp16, via fp32r matmuls)
  attn:  per head-pair: S^T = K^T.T Q^T (fp16 matmuls, scores transposed so the
         softmax axis lands on the PSUM free dim); P^T = exp(S^T/8) on ScalarE
         (no max subtraction needed: |S/8| < ~6); O_aug^T = V_aug^T P^T
         accumulated in PSUM, row 64 = softmax denominators; normalization on
         DVE + GpSimd (reciprocal + partition broadcast + multiply) so the PE
         never waits on it. The QKV projection matmuls for the NEXT head pair
         are interleaved into the attention loop: attention is ScalarE
         (exp)-bound, so the spare PE cycles compute the next pair's Q^T/K^T
         for free.
  tail:  partial out = O^T.T w_out (fp32r) -> DRAM

Matmul dtypes: fp32r (tf32-like, full speed) for QKV/out-proj where operand
precision matters most; fp16 for S/PV (same PE cycles, but halves SBUF traffic
and weight-load time; adds ~5e-4 operand rounding).
"""
import sys

sys.path.insert(0, "/opt/trn_rl_repo")

import numpy as np

import concourse.bacc as bacc
import concourse.mybir as mybir
from concourse import masks
from concourse.tile import TileContext
from concourse.bass_utils import run_bass_kernel_spmd

F32 = mybir.dt.float32
F32R = mybir.dt.float32r
F16 = mybir.dt.float16
EXP = mybir.ActivationFunctionType.Exp

T = 2048      # tokens per core (one batch element)
E = 1024      # model dim
HPC = 8       # heads per core
D = 64        # head dim
SCALE = D ** -0.5
NEC = E // 128   # 8 e-chunks
NI = 4           # i blocks of 512 (attention query cols)
NJ = 16          # j blocks of 128 (attention key rows = t blocks)

_CACHED_NC = None


def build_nc():
    nc = bacc.Bacc("TRN2", target_bir_lowering=False, debug=False, num_devices=8)
    x_d = nc.declare_dram_parameter("x", [T, E], F32R, isOutput=False)
    wqk_d = nc.declare_dram_parameter("wqk", [E, 1024], F32R, isOutput=False)
    wv_d = nc.declare_dram_parameter("wv", [E, 512], F32R, isOutput=False)
    wo_d = nc.declare_dram_parameter("wo", [512, E], F32R, isOutput=False)
    out_d = nc.declare_dram_parameter("out", [T, E], F32, isOutput=True)

    with TileContext(nc) as tc:
        with (
            tc.tile_pool(name="const", bufs=1) as const_pool,
            tc.tile_pool(name="qkt", bufs=2) as qkt_pool,
            tc.tile_pool(name="vaugp", bufs=1) as vaug_pool,
            tc.tile_pool(name="xph", bufs=1) as x_pool,
            tc.tile_pool(name="wstr", bufs=4) as w_pool,
        ):
            identF = const_pool.tile([128, 128], F32, tag="identF")
            masks.make_identity(nc, identF[:])
            ident = const_pool.tile([128, 128], F32R, tag="ident")
            nc.vector.tensor_copy(ident[:], identF[:])
            onesF = const_pool.tile([128, 64], F32, tag="onesF")
            nc.vector.memset(onesF[:], 1.0)

            vaug = [
                vaug_pool.tile([128, HPC * 65], F16, tag=f"va{jb}", name=f"va{jb}")
                for jb in range(NJ)
            ]
            xT = [
                x_pool.tile([128, T], F32R, tag=f"xT{ec}", name=f"xT{ec}")
                for ec in range(NEC)
            ]

            def qk_pair_closures(pair, qp, kp, psum_ref):
                """Closures computing Q^T/K^T chunks for head pair `pair` into
                qp/kp (fp16). First closure prefetches the weights. psum_ref is
                a 1-element list so deferred closures allocate from whichever
                PSUM pool is open when they actually run."""
                wq = w_pool.tile([128, E], F32R, tag="wcb", name=f"wq{pair}")
                wk = w_pool.tile([128, E], F32R, tag="wcb", name=f"wk{pair}")
                cls = []

                def load_w():
                    for ec in range(NEC):
                        nc.gpsimd.dma_start(
                            out=wq[:, ec * 128 : (ec + 1) * 128],
                            in_=wqk_d[
                                ec * 128 : (ec + 1) * 128,
                                pair * 128 : (pair + 1) * 128,
                            ],
                        )
                        nc.gpsimd.dma_start(
                            out=wk[:, ec * 128 : (ec + 1) * 128],
                            in_=wqk_d[
                                ec * 128 : (ec + 1) * 128,
                                512 + pair * 128 : 512 + (pair + 1) * 128,
                            ],
                        )

                cls.append(load_w)
                for wcb, dst in ((wq, qp), (wk, kp)):
                    for ib in range(NI):
                        def grp(wcb=wcb, dst=dst, ib=ib):
                            ps = psum_ref[0].tile([128, 512], F32, tag="qkp")
                            for ec in range(NEC):
                                nc.tensor.matmul(
                                    ps[:],
                                    wcb[:, ec * 128 : (ec + 1) * 128],
                                    xT[ec][:, ib * 512 : (ib + 1) * 512],
                                    start=(ec == 0),
                                    stop=(ec == NEC - 1),
                                )
                            nc.vector.tensor_copy(
                                dst[:, ib * 512 : (ib + 1) * 512], ps[:]
                            )

                        cls.append(grp)
                return cls

            # ---------------- head phase: x^T, V_aug, qk pair 0 ------------
            with (
                tc.tile_pool(name="xst", bufs=6) as xst_pool,
                tc.tile_pool(name="wvp", bufs=1) as wv_pool,
                tc.tile_pool(name="tp_ps", bufs=3, space="PSUM") as tp_psum,
                tc.tile_pool(name="qk_ps", bufs=3, space="PSUM") as qk_psum,
            ):
                xts = []
                for tb in range(16):
                    xt = xst_pool.tile([128, E], F32R, tag="xstage", name=f"xt{tb}")
                    trows = slice(tb * 128, (tb + 1) * 128)
                    nc.sync.dma_start(out=xt[:, 0:512], in_=x_d[trows, 0:512])
                    nc.sync.dma_start(out=xt[:, 512:1024], in_=x_d[trows, 512:1024])
                    xts.append(xt)
                wv_sb = wv_pool.tile([128, NEC * 512], F32R, tag="wv")
                for ec in range(NEC):
                    nc.gpsimd.dma_start(
                        out=wv_sb[:, ec * 512 : (ec + 1) * 512],
                        in_=wv_d[ec * 128 : (ec + 1) * 128, :],
                    )

                # x^T via PE transpose of 128x128 blocks, with each t-block's
                # V matmul group woven in one step behind its transposes so V
                # rides the DMA-paced gaps instead of serializing after.
                def emit_v(jb):
                    vview = vaug[jb][:].rearrange("p (h c) -> p h c", c=65)
                    nc.vector.tensor_copy(
                        vview[:, :, 64:65],
                        onesF[:, 0:HPC].rearrange("p (h c) -> p h c", c=1),
                    )
                    ps = qk_psum.tile([128, 512], F32, tag="qkp")
                    for ec in range(NEC):
                        nc.tensor.matmul(
                            ps[:],
                            xT[ec][:, jb * 128 : (jb + 1) * 128],
                            wv_sb[:, ec * 512 : (ec + 1) * 512],
                            start=(ec == 0),
                            stop=(ec == NEC - 1),
                        )
                    nc.vector.tensor_copy(
                        vview[:, :, 0:64], ps[:].rearrange("p (h c) -> p h c", c=64)
                    )

                for tb in range(16):
                    for ec in range(NEC):
                        pst = tp_psum.tile([128, 128], F32, tag="tp")
                        nc.tensor.transpose(
                            pst[:].bitcast(F32R),
                            xts[tb][:, ec * 128 : (ec + 1) * 128],
                            ident[:],
                        )
                        dstap = xT[ec][:, tb * 128 : (tb + 1) * 128]
                        if (tb + ec) % 2 == 0:
                            nc.vector.tensor_copy(dstap, pst[:])
                        else:
                            nc.scalar.copy(dstap, pst[:])
                    if tb > 0:
                        emit_v(tb - 1)
                emit_v(15)

                # qk chunks for pair 0: weights, q-ib0 and all k-groups
                # upfront (every k column is swept within the first j-loop);
                # q-ib1..3 deferred into hc0's attention loop.
                qp0 = qkt_pool.tile([128, T], F16, tag="qp", name="qp0")
                kp0 = qkt_pool.tile([128, T], F16, tag="kp", name="kp0")
                qk0_psum_ref = [qk_psum]
                cls0 = qk_pair_closures(0, qp0, kp0, qk0_psum_ref)
                cls0[0]()          # load_w
                cls0[1]()          # q-ib0
                for fn in cls0[5:9]:
                    fn()           # k-ib0..3
                qk0_deferred = cls0[2:5]

            # ---------------- attention + interleaved next-pair QKV --------
            with (
                tc.tile_pool(name="otp", bufs=1) as ot_pool,
                tc.tile_pool(name="wop", bufs=1) as wo_pool,
            ):
                oT = [
                    ot_pool.tile([128, T], F32R, tag=f"oT{hc}", name=f"oT{hc}")
                    for hc in range(4)
                ]
                wo_sb = [
                    wo_pool.tile([128, E], F32R, tag=f"wo{hc}", name=f"wo{hc}")
                    for hc in range(4)
                ]
                for hc in range(4):
                    nc.gpsimd.dma_start(
                        out=wo_sb[hc][:], in_=wo_d[hc * 128 : (hc + 1) * 128, :]
                    )

                with (
                    tc.tile_pool(name="pt", bufs=3) as pt_pool,
                    tc.tile_pool(name="ocp", bufs=3) as oc_pool,
                    tc.tile_pool(name="rbp", bufs=3) as rb_pool,
                    tc.tile_pool(name="s_ps", bufs=2, space="PSUM") as s_psum,
                    tc.tile_pool(name="oa_ps", bufs=2, space="PSUM") as oa_psum,
                    tc.tile_pool(name="qk3_ps", bufs=2, space="PSUM") as qk3_psum,
                ):
                    qp, kp = qp0, kp0
                    pending = []
                    for hc in range(4):
                        hA, hB = 2 * hc, 2 * hc + 1
                        if hc < 3:
                            qn = qkt_pool.tile([128, T], F16, tag="qp", name=f"qp{hc+1}")
                            kn = qkt_pool.tile([128, T], F16, tag="kp", name=f"kp{hc+1}")
                            pending = qk_pair_closures(hc + 1, qn, kn, [qk3_psum])
                            if hc == 0:
                                qk0_psum_ref[0] = qk3_psum
                                pending = qk0_deferred + pending
                        else:
                            qn = kn = None
                        steps = 0
                        for ib in range(NI):
                            icols = slice(ib * 512, (ib + 1) * 512)
                            oaugA = oa_psum.tile([65, 512], F32, tag="oa", name="oaugA")
                            oaugB = oa_psum.tile([65, 512], F32, tag="oa", name="oaugB")
                            prev_pAB = None

                            def emit_pv(pAB, jb):
                                nc.tensor.matmul(
                                    oaugA[:],
                                    vaug[jb][:, hA * 65 : hA * 65 + 65],
                                    pAB[:, 0:512],
                                    start=(jb == 0), stop=(jb == NJ - 1),
                                )
                                nc.tensor.matmul(
                                    oaugB[:],
                                    vaug[jb][:, hB * 65 : hB * 65 + 65],
                                    pAB[:, 512:1024],
                                    start=(jb == 0), stop=(jb == NJ - 1),
                                )

                            for jb in range(NJ):
                                jcols = slice(jb * 128, (jb + 1) * 128)
                                sAB = s_psum.tile([128, 1024], F32, tag="sAB")
                                nc.tensor.matmul(
                                    sAB[:, 0:512], kp[0:64, jcols], qp[0:64, icols],
                                    start=True, stop=True,
                                )
                                nc.tensor.matmul(
                                    sAB[:, 512:1024], kp[64:128, jcols],
                                    qp[64:128, icols],
                                    start=True, stop=True,
                                )
                                pAB = pt_pool.tile([128, 1024], F16, tag="pAB")
                                nc.scalar.activation(pAB[:], sAB[:], EXP, scale=SCALE)
                                if prev_pAB is not None:
                                    emit_pv(prev_pAB, jb - 1)
                                prev_pAB = pAB
                                # weave next pair's QKV into spare PE cycles
                                steps += 1
                                if pending and steps % 5 == 0:
                                    pending.pop(0)()
                            emit_pv(prev_pAB, NJ - 1)

                            for oaug, rowoff in ((oaugA, 0), (oaugB, 64)):
                                oc = oc_pool.tile([65, 512], F32, tag="oc")
                                nc.vector.tensor_copy(oc[:], oaug[:])
                                rc0 = oc_pool.tile([1, 512], F32, tag="rc0")
                                nc.vector.reciprocal(rc0[0:1, :], oc[64:65, :])
                                rbs = rb_pool.tile([64, 512], F32, tag="rbs")
                                nc.gpsimd.partition_broadcast(rbs[:], rc0[0:1, :])
                                nc.vector.tensor_mul(
                                    oT[hc][rowoff : rowoff + 64, icols],
                                    oc[0:64, :],
                                    rbs[:],
                                )
                        for fn in pending:
                            fn()
                        pending = []
                        qp, kp = qn, kn


                # ---------------- tail: output projection ------------------
                with (
                    tc.tile_pool(name="prj_ps", bufs=3, space="PSUM") as prj_psum,
                    tc.tile_pool(name="ost2", bufs=3) as out2_pool,
                ):
                    for tb in range(16):
                        trows = slice(tb * 128, (tb + 1) * 128)
                        for eb in range(2):
                            ecols = slice(eb * 512, (eb + 1) * 512)
                            ps = prj_psum.tile([128, 512], F32, tag="prj")
                            for hc in range(4):
                                nc.tensor.matmul(
                                    ps[:],
                                    oT[hc][:, trows],
                                    wo_sb[hc][:, ecols],
                                    start=(hc == 0),
                                    stop=(hc == 3),
                                )
                            ot = out2_pool.tile([128, 512], F32, tag="ost")
                            nc.scalar.copy(ot[:], ps[:])
                            eng = nc.sync if (tb + eb) % 2 == 0 else nc.gpsimd
                            eng.dma_start(out=out_d[trows, ecols], in_=ot[:])

    nc.compile()
    return nc


def get_nc():
    global _CACHED_NC
    if _CACHED_NC is None:
        _CACHED_NC = build_nc()
    return _CACHED_NC


def make_in_maps(x, w_qkv, w_out):
    in_maps = []
    for c in range(8):
        bi, hg = divmod(c, 2)
        wqk_c = np.concatenate(
            [
                w_qkv[:, hg * 512 : hg * 512 + 512],
                w_qkv[:, 1024 + hg * 512 : 1024 + hg * 512 + 512],
            ],
            axis=1,
        )
        in_maps.append(
            {
                "x": np.ascontiguousarray(x[bi]),
                "wqk": np.ascontiguousarray(wqk_c),
                "wv": np.ascontiguousarray(
                    w_qkv[:, 2048 + hg * 512 : 2048 + hg * 512 + 512]
                ),
                "wo": np.ascontiguousarray(w_out[hg * 512 : hg * 512 + 512, :]),
            }
        )
    return in_maps


def kernel(x, w_qkv, w_out, b_out):
    x = np.asarray(x, dtype=np.float32)
    w_qkv = np.asarray(w_qkv, dtype=np.float32)
    w_out = np.asarray(w_out, dtype=np.float32)
    b_out = np.asarray(b_out, dtype=np.float32)
    nc = get_nc()
    res = run_bass_kernel_spmd(nc, make_in_maps(x, w_qkv, w_out), list(range(8)))
    parts = [res.results[c]["out"] for c in range(8)]
    out = np.stack([parts[2 * bi] + parts[2 * bi + 1] for bi in range(4)])
    out += b_out[None, None, :]
    return out.astype(np.float32)

